# revision 42
# baseline (speedup 1.0000x reference)
"""Trainium2 Bass kernel for nn_Attention_33921651703853 (sparse_attention).

Data-parallel over batch: B=256 -> 32 batches on each of 8 NeuronCores.
All weights replicated; no collectives.

v2 design (PE-bound baseline at 1.15ms, PE 95% busy):
  - q/k projection in fp8e4 (e4m3) with DoubleRow perf mode: contracts 256
    rows per matmul at 2x bf16 throughput. Scales: Wq' = Wq*scale*1024,
    Wk' = Wk*32; the combined 2^15 factor is divided out by the activation
    `scale` at exp/relu evacuation time. v stays bf16 (fp8 noise on v would
    land directly in the output).
  - Stationary-operand reuse everywhere: each ldweights feeds 2 matmuls
    (two psum banks accumulate in parallel), halving LDWEIGHTS pressure.
  - Attention processed in head PAIRS: all DVE/Act elementwise work is
    [_, 2*...] wide, halving per-instruction fixed costs. Scores stay in
    PSUM through the keypoint-MLP correction (added in place by DVE);
    exp reads PSUM directly with the mask as per-partition bias and the
    fp8 descale as activation scale. No separate score materialization.
  - v projection tails (tokens 128:164 of each batch) are computed two
    batches per stationary ([128,2,36] AP), then split to their per-batch
    SBUF homes by SBUF->SBUF DMA (partition shift).
  - Final projection computes yT[dout, i] with wout as stationary and the
    group outT [128, 8*656] as moving: no ragged-i waste, bout added as
    per-partition bias during PSUM evacuation; host undoes the transpose.
  - Per-group software pipeline: group g's attention zips group g+1's
    projection chains and group g-1's final chains between dependency
    stages, keeping TensorE dense (full HAM clock).
"""

import os
import numpy as np
import ml_dtypes

import concourse.bass as bass
import concourse.bacc as bacc
import concourse.bass_isa as bass_isa
import concourse.mybir as mybir
import concourse.tile as tile
from concourse import bass_utils

BF16 = mybir.dt.bfloat16
F32 = mybir.dt.float32
F8 = mybir.dt.float8e4
AF = mybir.ActivationFunctionType
ALU = mybir.AluOpType
DR = mybir.MatmulPerfMode.DoubleRow
nbf16 = ml_dtypes.bfloat16
nf8 = ml_dtypes.float8_e4m3

B, N, D, H, NK, DH = 256, 164, 1024, 8, 100, 128
NCORES = 8
BPC = B // NCORES          # 32 batches per core
GB = 4                     # batches per group
NG = BPC // GB             # 8 groups
XC = GB * N                # 656 tokens per group
N1 = N - 128               # 36
MASK_NEG = -1.0e30
SQK = float(2 ** 15)       # Sq*Sk fp8 pre-scale (descaled at activation)

_CACHE = {}
LAST_EXEC_NS = None


def _install_profile_hook():
    """Make run_bass_kernel_spmd(trace=True) work under axon in this image."""
    import sys as _sys
    import types as _types
    try:
        import antenv  # noqa: F401
        try:
            from antenv.axon_hooks import get_axon_ntff_profile_hook  # noqa: F401
        except ImportError:
            from trn_agent_boot.trn_boot import _ntff_profile_via_ctypes
            hook = _ntff_profile_via_ctypes("/opt/axon/libaxon_pjrt.so")
            mod = _types.ModuleType("antenv.axon_hooks")
            mod._hook = hook
            mod.set_axon_ntff_profile_hook = lambda h: setattr(mod, "_hook", h)
            mod.get_axon_ntff_profile_hook = lambda: mod._hook
            _sys.modules["antenv.axon_hooks"] = mod
            antenv.axon_hooks = mod
        if not getattr(bass_utils, "_upload_patched", False):
            _orig_upload = bass_utils.upload_artifacts

            def _safe_upload(tmpdir):
                try:
                    return _orig_upload(tmpdir)
                except Exception:
                    return tmpdir

            bass_utils.upload_artifacts = _safe_upload
            bass_utils._upload_patched = True
        return True
    except Exception as e:  # pragma: no cover
        print(f"profile hook install failed: {type(e).__name__}: {e}")
        return False


def _build_nc():
    nc = bacc.Bacc("TRN2", target_bir_lowering=False, debug=False)

    # ---- DRAM parameters (per-core shapes) ----
    d_xtv = nc.dram_tensor("xtv", [NG, 128, 8, 768], BF16, kind="ExternalInput")
    d_xt8 = nc.dram_tensor("xt8", [NG, 128, 8, XC], F8, kind="ExternalInput")
    d_wqk8 = nc.dram_tensor("wqk8", [128, 8, 2 * D], F8, kind="ExternalInput")
    d_wv = nc.dram_tensor("wv", [128, 8, D], BF16, kind="ExternalInput")
    d_wout = nc.dram_tensor("wout", [128, 8 * D], BF16, kind="ExternalInput")
    d_w1 = nc.dram_tensor("w1", [NK, 50], BF16, kind="ExternalInput")
    d_w2 = nc.dram_tensor("w2", [50, NK], BF16, kind="ExternalInput")
    d_b1 = nc.dram_tensor("b1c", [50, 1], F32, kind="ExternalInput")
    d_b2 = nc.dram_tensor("b2c", [NK, 1], F32, kind="ExternalInput")
    d_boutT = nc.dram_tensor("boutT", [128, 8], F32, kind="ExternalInput")
    d_mbt0 = nc.dram_tensor("mbt0", [128, BPC], F32, kind="ExternalInput")
    d_mbt1 = nc.dram_tensor("mbt1", [N1, BPC], F32, kind="ExternalInput")
    d_xian2 = nc.dram_tensor("xian2", [NG, NK, 2 * GB, NK], BF16, kind="ExternalInput")
    d_y = nc.dram_tensor("y", [8, 128, BPC * N], BF16, kind="ExternalOutput")

    from contextlib import ExitStack
    with tile.TileContext(nc) as tc, ExitStack() as es:
        ec = es.enter_context
        cpool = ec(tc.tile_pool(name="const", bufs=1))
        xt_pool = ec(tc.tile_pool(name="xt", bufs=2))
        xt8_pool = ec(tc.tile_pool(name="xt8", bufs=2))
        xian_pool = ec(tc.tile_pool(name="xian", bufs=2))
        qk_pool = ec(tc.tile_pool(name="qk", bufs=2))
        v_pool = ec(tc.tile_pool(name="vsb", bufs=2))
        vtmp_pool = ec(tc.tile_pool(name="vtmp", bufs=2))
        outT_pool = ec(tc.tile_pool(name="outT", bufs=2))
        y_pool = ec(tc.tile_pool(name="ysb", bufs=2))
        rbc_pool = ec(tc.tile_pool(name="rbc", bufs=2))
        small_pool = ec(tc.tile_pool(name="smallsb", bufs=3))
        # PSUM is 8 banks; every tile slot is a full bank. 3 (proj ring)
        # + 2 (score-full) + 1 (score-tail + denom share a ring) + 2
        # (m1/m2/oT share a ring) = 8.
        pj = ec(tc.tile_pool(name="pproj", bufs=3, space="PSUM"))
        pscf = ec(tc.tile_pool(name="pscf", bufs=2, space="PSUM"))
        psct = ec(tc.tile_pool(name="psct", bufs=1, space="PSUM"))
        pmix = ec(tc.tile_pool(name="pmix", bufs=2, space="PSUM"))
        if True:
            # ---- constants ----
            wqk8_sb = cpool.tile([128, 8, 2 * D], F8, tag="wqk8")
            wv_sb = cpool.tile([128, 8, D], BF16, tag="wv")
            wout_sb = cpool.tile([128, 8 * D], BF16, tag="wout")
            w1_sb = cpool.tile([NK, 50], BF16, tag="w1")
            w2_sb = cpool.tile([50, NK], BF16, tag="w2")
            b1_sb = cpool.tile([50, 1], F32, tag="b1")
            b2_sb = cpool.tile([NK, 1], F32, tag="b2")
            boutT_sb = cpool.tile([128, 8], F32, tag="boutT")
            mbt0_sb = cpool.tile([128, BPC], F32, tag="mbt0")
            mbt1_sb = cpool.tile([N1, BPC], F32, tag="mbt1")
            onesm_sb = cpool.tile([128, 128], BF16, tag="onesm")
            nc.vector.memset(onesm_sb[:], 1.0)


            def load_consts():
                # wqk8 split per channel-tile pair so early qk chains can start
                for ct in range(8):
                    nc.sync.dma_start(
                        wqk8_sb[:, :, ct * 256:(ct + 1) * 256],
                        d_wqk8.ap()[:, :, ct * 256:(ct + 1) * 256])
                nc.sync.dma_start(wv_sb[:], d_wv.ap()[:, :, :])
                nc.sync.dma_start(w1_sb[:], d_w1.ap()[:, :])
                nc.sync.dma_start(w2_sb[:], d_w2.ap()[:, :])
                nc.sync.dma_start(b1_sb[:], d_b1.ap()[:, :])
                nc.sync.dma_start(b2_sb[:], d_b2.ap()[:, :])
                nc.sync.dma_start(mbt0_sb[:], d_mbt0.ap()[:, :])
                nc.sync.dma_start(mbt1_sb[:], d_mbt1.ap()[:, :])
                nc.sync.dma_start(wout_sb[:], d_wout.ap()[:, :])
                nc.sync.dma_start(boutT_sb[:], d_boutT.ap()[:, :])

            group_tiles = {}

            def start_group(g):
                """DMA group g's inputs; return list of projection closures."""
                xt8_sb = xt8_pool.tile([128, 8, XC], F8, tag="xt8")
                nc.sync.dma_start(xt8_sb[:], d_xt8.ap()[g, :, :, :])
                xtv_sb = xt_pool.tile([128, 8, 768], BF16, tag="xtv")
                nc.sync.dma_start(xtv_sb[:], d_xtv.ap()[g, :, :, :])
                xian_sb = xian_pool.tile([NK, 2 * GB, NK], BF16, tag="xian")
                nc.sync.dma_start(xian_sb[:], d_xian2.ap()[g, :, :, :])
                qkT = qk_pool.tile([128, 16 * XC], BF16, tag="qkT")
                v_sb = v_pool.tile([128, GB * 2 * D], BF16, tag="v")
                group_tiles[g] = (xtv_sb, xt8_sb, qkT, v_sb, xian_sb)

                def qk_chain(ct):
                    # fp8 DoubleRow: one stationary pair feeds both token
                    # halves; 4 chained matmuls cover the K=1024 contraction.
                    # Split into 2 sub-closures for finer zip interleaving.
                    st8 = {}

                    def half(lo):
                        if lo == 0:
                            st8["pa"] = pj.tile([128, 512], F32, tag="pj", name="qkpa")
                            st8["pb"] = pj.tile([128, 512], F32, tag="pj", name="qkpb")
                        pa, pb = st8["pa"], st8["pb"]
                        for p in range(lo, lo + 2):
                            st = wqk8_sb[:, 2 * p:2 * p + 2, ct * 128:ct * 128 + 128]
                            nc.tensor.matmul(
                                pa[:, :], st, xt8_sb[:, 2 * p:2 * p + 2, 0:512],
                                start=(p == 0), stop=(p == 3), perf_mode=DR)
                            nc.tensor.matmul(
                                pb[:, :144], st, xt8_sb[:, 2 * p:2 * p + 2, 512:XC],
                                start=(p == 0), stop=(p == 3), perf_mode=DR)
                        if lo == 2:
                            nc.scalar.activation(qkT[:, ct * XC:ct * XC + 512],
                                                 pa[:], AF.Copy)
                            nc.scalar.activation(qkT[:, ct * XC + 512:ct * XC + XC],
                                                 pb[:, :144], AF.Copy)
                    return [lambda: half(0), lambda: half(2)]

                def v_chain(s):
                    # xtv stationary tiles: s in 0..3 are batch s tokens
                    # 0:128; s==4 is the packed tails (b0,b1,b2 full + b3's
                    # first 20); s==5 is b3's last 16 tail tokens (+zero pad).
                    # Split into 4 sub-closures of 2 dt-steps each.
                    stv = {}
                    pw = 128 if s < 5 else 16

                    def quarter(lo):
                        if lo == 0:
                            stv["pa"] = pj.tile([128, 512], F32, tag="pj", name="vpa")
                            stv["pb"] = pj.tile([128, 512], F32, tag="pj", name="vpb")
                        pa, pb = stv["pa"], stv["pb"]
                        for dt in range(lo, lo + 2):
                            st = xtv_sb[:, dt, s * 128:s * 128 + 128]
                            nc.tensor.matmul(pa[:pw, :], st[:, :pw],
                                             wv_sb[:, dt, 0:512],
                                             start=(dt == 0), stop=(dt == 7))
                            nc.tensor.matmul(pb[:pw, :], st[:, :pw],
                                             wv_sb[:, dt, 512:D],
                                             start=(dt == 0), stop=(dt == 7))
                        if lo == 6:
                            v_evac(s, pa, pb, pw)

                    return [lambda q=q: quarter(2 * q) for q in range(4)]

                def v_evac(s, pa, pb, pw):
                    if s < 4:
                        base = (s * 2) * D
                        nc.vector.tensor_copy(v_sb[:128, base:base + 512], pa[:128, :])
                        nc.vector.tensor_copy(v_sb[:128, base + 512:base + D],
                                              pb[:128, :])
                    elif s == 4:
                        vt = vtmp_pool.tile([128, D], BF16, tag="vt")
                        nc.vector.tensor_copy(vt[:, 0:512], pa[:, :])
                        nc.vector.tensor_copy(vt[:, 512:D], pb[:, :])
                        # scatter packed tails to per-batch homes
                        for bb in range(3):
                            nc.sync.dma_start(
                                v_sb[0:N1, (bb * 2 + 1) * D:(bb * 2 + 2) * D],
                                vt[bb * N1:(bb + 1) * N1, :])
                        nc.sync.dma_start(
                            v_sb[0:20, 7 * D:8 * D], vt[108:128, :])
                    else:
                        vt = vtmp_pool.tile([128, D], BF16, tag="vt2")
                        nc.vector.tensor_copy(vt[:16, 0:512], pa[:16, :])
                        nc.vector.tensor_copy(vt[:16, 512:D], pb[:16, :])
                        nc.sync.dma_start(
                            v_sb[20:N1, 7 * D:8 * D], vt[0:16, :])

                qk_parts, v_parts = [], []
                for i in range(8):
                    qk_parts += qk_chain(i)
                    qk_parts += qk_chain(8 + i)
                    if i < 6:
                        v_parts += v_chain(i)
                chains = []
                for i in range(8):
                    chains += qk_parts[4 * i:4 * i + 4]
                    if i < 6:
                        chains += v_parts[4 * i:4 * i + 4]
                # prologue variant: qk first (xt8 lands before xtv)
                return chains, qk_parts + v_parts

            def final_chain(dout, outT_prev, gp):
                # split into 4 sub-closures of 2 h-steps each
                stf = {}

                def part(lo):
                    if lo == 0:
                        stf["pa"] = pj.tile([128, 512], F32, tag="pj", name="fpa")
                        stf["pb"] = pj.tile([128, 512], F32, tag="pj", name="fpb")
                    pa, pb = stf["pa"], stf["pb"]
                    for h in range(lo, lo + 2):
                        st = wout_sb[:, h * D + dout * 128:h * D + dout * 128 + 128]
                        nc.tensor.matmul(pa[:, :328], st,
                                         outT_prev[:, h, 0:328],
                                         start=(h == 0), stop=(h == 7))
                        nc.tensor.matmul(pb[:, :328], st,
                                         outT_prev[:, h, 328:XC],
                                         start=(h == 0), stop=(h == 7))
                    if lo == 6:
                        y_sb = y_pool.tile([128, XC], BF16, tag="y")
                        nc.scalar.activation(y_sb[:, 0:328], pa[:, :328],
                                             AF.Identity,
                                             bias=boutT_sb[:, dout:dout + 1])
                        nc.scalar.activation(y_sb[:, 328:XC], pb[:, :328],
                                             AF.Identity,
                                             bias=boutT_sb[:, dout:dout + 1])
                        nc.sync.dma_start(
                            d_y.ap()[dout, :, gp * XC:(gp + 1) * XC], y_sb[:])

                return [lambda p=p: part(2 * p) for p in range(4)]

            def final_chain_b(dout, outT_prev, gp, b):
                # single-batch final (last group: overlap drain with attention)
                pa = pj.tile([128, 512], F32, tag="pj")
                for h in range(8):
                    st = wout_sb[:, h * D + dout * 128:h * D + dout * 128 + 128]
                    nc.tensor.matmul(pa[:, :N], st,
                                     outT_prev[:, h, b * N:b * N + N],
                                     start=(h == 0), stop=(h == 7))
                y_sb = y_pool.tile([128, N], BF16, tag="yb")
                nc.scalar.activation(y_sb[:], pa[:, :N], AF.Identity,
                                     bias=boutT_sb[:, dout:dout + 1])
                nc.sync.dma_start(
                    d_y.ap()[dout, :, gp * XC + b * N:gp * XC + (b + 1) * N],
                    y_sb[:])

            # ---- prologue ----
            # static probs ring: tail rows 36:128 (cols 328:656) are zeroed
            # once and never rewritten, so the Pool all-reduce over the full
            # tile sums full+tail parts without per-pair masking
            pc_ring = []
            for ri in range(3):
                pcr = cpool.tile([128, XC], BF16, tag=f"pc{ri}",
                                 name=f"pcr{ri}")
                nc.vector.memset(pcr[:], 0.0)
                pc_ring.append(pcr)
            pair_seq = [0]
            _, g0_chains = start_group(0)
            load_consts()
            for ch in g0_chains:
                ch()

            pending_final = None  # (outT, g)
            inv = 1.0 / SQK

            for g in range(NG):
                xtv_sb, xt8_sb, qkT, v_sb, xian_sb = group_tiles.pop(g)
                zlist = []
                if g + 1 < NG:
                    zlist += start_group(g + 1)[0]
                if pending_final is not None:
                    of, gp = pending_final
                    for dout in range(8):
                        zlist += final_chain(dout, of, gp)
                pending_final = None

                outT = outT_pool.tile([128, 8, XC], BF16, tag="outT")
                zi = 0          # next zip item
                steps = 0       # zip-points passed (4 per pair)
                ztot = 4 * GB * 4

                def zip_step():
                    nonlocal zi, steps
                    steps += 1
                    while zi < (len(zlist) * steps) // ztot:
                        zlist[zi]()
                        zi += 1

                for b in range(GB):
                    gb = g * GB + b
                    for hp in range(4):
                        h0 = 2 * hp
                        # ---- scores (PSUM-resident) ----
                        scf = pscf.tile([128, 2, N], F32, tag="scf")
                        sct = psct.tile([128, 2 * N], F32, tag="sct")
                        for i in range(2):
                            qof = (h0 + i) * XC + b * N
                            kof = (8 + h0 + i) * XC + b * N
                            nc.tensor.matmul(scf[:, i, :], qkT[:, kof:kof + 128],
                                             qkT[:, qof:qof + N])
                            nc.tensor.matmul(sct[:N1, i * N:i * N + N],
                                             qkT[:, kof + 128:kof + N],
                                             qkT[:, qof:qof + N])
                        zip_step()
                        # ---- keypoint MLP (both heads batched) ----
                        raq = small_pool.tile([NK, 2, NK], BF16, tag="raq")
                        nc.scalar.activation(raq[:], scf[:NK, :, :NK], AF.Relu,
                                             scale=inv)
                        mx1 = pmix.tile([128, 2 * N], F32, tag="mix")
                        m1 = mx1[:50, :2 * NK]
                        nc.tensor.matmul(m1, w1_sb[:], raq[:])
                        zip_step()
                        h1t = small_pool.tile([50, 2 * NK], BF16, tag="h1")
                        nc.scalar.activation(h1t[:], m1, AF.Relu,
                                             bias=b1_sb[:])
                        mx2 = pmix.tile([128, 2 * N], F32, tag="mix")
                        m2 = mx2[:NK, :2 * NK]
                        nc.tensor.matmul(m2, w2_sb[:], h1t[:])
                        zip_step()
                        lvt = small_pool.tile([NK, 2 * NK], BF16, tag="lv")
                        nc.vector.tensor_scalar(lvt[:], m2, b2_sb[:], 0.0,
                                                ALU.add, ALU.max)
                        tmp = small_pool.tile([NK, 2 * NK], F32, tag="tmp")
                        nc.vector.tensor_mul(tmp[:], xian_sb[:, 2 * b:2 * b + 2, :],
                                             lvt[:])
                        nc.vector.tensor_add(scf[:NK, :, :NK], scf[:NK, :, :NK],
                                             tmp[:])
                        # ---- exp straight out of PSUM (mask bias, descale);
                        # full probs in cols 0:328, tail probs in cols
                        # 328:656 rows 0:36 (rows 36:128 pre-zeroed) ----
                        pc = pc_ring[pair_seq[0] % 3]
                        pair_seq[0] += 1
                        nc.scalar.activation(pc[:, 0:2 * N], scf[:], AF.Exp,
                                             bias=mbt0_sb[:, gb:gb + 1], scale=inv)
                        nc.scalar.activation(pc[0:N1, 2 * N:XC], sct[:N1], AF.Exp,
                                             bias=mbt1_sb[:, gb:gb + 1], scale=inv)
                        # ---- denominator (ones-matmul partition broadcast) ----
                        db = psct.tile([128, 2 * N], F32, tag="sct")
                        nc.tensor.matmul(db[:], onesm_sb[:], pc[:, 0:2 * N],
                                         start=True, stop=False)
                        nc.tensor.matmul(db[:], onesm_sb[:N1, :],
                                         pc[0:N1, 2 * N:XC],
                                         start=False, stop=True)
                        rb = rbc_pool.tile([128, 2 * N], F32, tag="rb")
                        nc.vector.reciprocal_approx_fast(rb[:], db[:])
                        # ---- AV ----
                        ot = pmix.tile([128, 2 * N], F32, tag="mix")
                        for i in range(2):
                            h = h0 + i
                            nc.tensor.matmul(
                                ot[:, i * N:i * N + N],
                                v_sb[:, (b * 2) * D + h * DH:(b * 2) * D + h * DH + DH],
                                pc[:, i * N:i * N + N], start=True, stop=False)
                            nc.tensor.matmul(
                                ot[:, i * N:i * N + N],
                                v_sb[:N1, (b * 2 + 1) * D + h * DH:(b * 2 + 1) * D + h * DH + DH],
                                pc[0:N1, 2 * N + i * N:2 * N + i * N + N],
                                start=False, stop=True)
                        nc.vector.tensor_mul(
                            outT[:, h0:h0 + 2, b * N:b * N + N], ot[:], rb[:])
                        zip_step()

                    if g == NG - 1:
                        for dout in range(8):
                            final_chain_b(dout, outT, g, b)

                while zi < len(zlist):
                    zlist[zi]()
                    zi += 1
                if g < NG - 1:
                    pending_final = (outT, g)

    nc.compile()
    return nc


def _prep_core_inputs(xc, maskc, xianc, consts):
    # (g, p, dt, b, n) layout, partition = d-within-dt
    xt = xc.transpose(0, 2, 1).reshape(NG, GB, 8, 128, N)
    xt = np.ascontiguousarray(xt.transpose(0, 3, 2, 1, 4))
    # xtv bf16: v-projection stationary tiles [NG, 128, 8, 6*128]:
    # tiles 0..3 = per-batch tokens 0:128; tile 4 = packed tails
    # (b0,b1,b2 all 36 + b3's first 20); tile 5 = b3 tail 20:36 + zeros.
    xtv = np.zeros((NG, 128, 8, 768), dtype=np.float32)
    for bb in range(GB):
        xtv[:, :, :, bb * 128:(bb + 1) * 128] = xt[:, :, :, bb, 0:128]
    tails = xt[:, :, :, :, 128:N]                    # [NG,128,8,GB,36]
    packed = tails.reshape(NG, 128, 8, GB * N1)      # b-major 144
    xtv[:, :, :, 512:640] = packed[:, :, :, 0:128]
    xtv[:, :, :, 640:656] = packed[:, :, :, 128:144]
    xtv = xtv.astype(nbf16)
    # xt8 fp8: [NG, 128, 8, 656] (tokens contiguous across batches)
    xt8 = xt.reshape(NG, 128, 8, XC).astype(nf8)
    # mask bias transposed: [164, BPC]
    mb = np.where(maskc, np.float32(MASK_NEG), np.float32(0.0)).astype(np.float32)
    mbt = np.ascontiguousarray(mb.T)
    # xian2: xianT scaled by SQK, duplicated per head-pair lane:
    # [NG, NK(j), 2*GB, NK(i)] where dim2 = (b, dup)
    xiant = (xianc.transpose(0, 2, 1) * np.float32(SQK)).reshape(NG, GB, NK, NK)
    xian2 = np.empty((NG, NK, GB, 2, NK), dtype=np.float32)
    xian2[:, :, :, 0, :] = xiant.transpose(0, 2, 1, 3)
    xian2[:, :, :, 1, :] = xiant.transpose(0, 2, 1, 3)
    xian2 = np.ascontiguousarray(xian2.reshape(NG, NK, 2 * GB, NK)).astype(nbf16)
    out = {
        "xtv": xtv,
        "xt8": xt8,
        "mbt0": np.ascontiguousarray(mbt[:128]),
        "mbt1": np.ascontiguousarray(mbt[128:]),
        "xian2": xian2,
    }
    out.update(consts)
    return out


def kernel(x, mask, xian, Wqkv, W1, b1, W2, b2, Wout, bout):
    global LAST_EXEC_NS
    x = np.asarray(x, dtype=np.float32)
    mask = np.asarray(mask)
    xian = np.asarray(xian, dtype=np.float32)
    Wqkv = np.asarray(Wqkv, dtype=np.float32)
    W1 = np.asarray(W1, dtype=np.float32)
    b1 = np.asarray(b1, dtype=np.float32)
    W2 = np.asarray(W2, dtype=np.float32)
    b2 = np.asarray(b2, dtype=np.float32)
    Wout = np.asarray(Wout, dtype=np.float32)
    bout = np.asarray(bout, dtype=np.float32)

    if "nc" not in _CACHE:
        _CACHE["nc"] = _build_nc()
    nc = _CACHE["nc"]

    # ---- shared weight prep ----
    scale = np.float32(D ** -0.5)
    # fp8 q/k weights with power-of-two pre-scales (descaled on device)
    wq = Wqkv[:, :D] * (scale * np.float32(1024.0))
    wk = Wqkv[:, D:2 * D] * np.float32(32.0)
    wqk = np.concatenate([wq, wk], axis=1)                    # [D, 2D]
    wqk8 = np.ascontiguousarray(
        wqk.reshape(8, 128, 2 * D).transpose(1, 0, 2)).astype(nf8)
    wv_h = np.ascontiguousarray(
        Wqkv[:, 2 * D:].reshape(8, 128, D).transpose(1, 0, 2)).astype(nbf16)
    wout_h = np.ascontiguousarray(
        Wout.reshape(8, 128, D).transpose(1, 0, 2)).reshape(128, 8 * D).astype(nbf16)
    consts = {
        "wqk8": wqk8,
        "wv": wv_h,
        "wout": wout_h,
        "w1": W1.astype(nbf16),
        "w2": W2.astype(nbf16),
        "b1c": np.ascontiguousarray(b1.reshape(50, 1)),
        "b2c": np.ascontiguousarray(b2.reshape(NK, 1)),
        "boutT": np.ascontiguousarray(bout.reshape(8, 128).T).astype(np.float32),
    }

    in_maps = []
    for c in range(NCORES):
        sl = slice(c * BPC, (c + 1) * BPC)
        in_maps.append(_prep_core_inputs(x[sl], mask[sl], xian[sl], consts))

    trace = bool(int(os.environ.get("KERNEL_TRACE", "0")))
    if trace:
        trace = _install_profile_hook()
    res = bass_utils.run_bass_kernel_spmd(
        nc, in_maps, core_ids=list(range(NCORES)), trace=trace)
    LAST_EXEC_NS = res.exec_time_ns
    _CACHE["res"] = res

    out = np.empty((B, N, D), dtype=np.float32)
    for c in range(NCORES):
        yc = res.results[c]["y"].astype(np.float32)           # [8, 128, BPC*N]
        yc = yc.reshape(8, 128, BPC, N).transpose(2, 3, 0, 1).reshape(BPC, N, D)
        out[c * BPC:(c + 1) * BPC] = yc
    return out


# revision 44
# speedup vs baseline: 1.0000x; 1.0000x over previous
"""Trainium2 Bass kernel for nn_Attention_33921651703853 (sparse_attention).

Data-parallel over batch: B=256 -> 32 batches on each of 8 NeuronCores.
All weights replicated; no collectives.

v2 design (PE-bound baseline at 1.15ms, PE 95% busy):
  - q/k projection in fp8e4 (e4m3) with DoubleRow perf mode: contracts 256
    rows per matmul at 2x bf16 throughput. Scales: Wq' = Wq*scale*1024,
    Wk' = Wk*32; the combined 2^15 factor is divided out by the activation
    `scale` at exp/relu evacuation time. v stays bf16 (fp8 noise on v would
    land directly in the output).
  - Stationary-operand reuse everywhere: each ldweights feeds 2 matmuls
    (two psum banks accumulate in parallel), halving LDWEIGHTS pressure.
  - Attention processed in head PAIRS: all DVE/Act elementwise work is
    [_, 2*...] wide, halving per-instruction fixed costs. Scores stay in
    PSUM through the keypoint-MLP correction (added in place by DVE);
    exp reads PSUM directly with the mask as per-partition bias and the
    fp8 descale as activation scale. No separate score materialization.
  - v projection tails (tokens 128:164 of each batch) are computed two
    batches per stationary ([128,2,36] AP), then split to their per-batch
    SBUF homes by SBUF->SBUF DMA (partition shift).
  - Final projection computes yT[dout, i] with wout as stationary and the
    group outT [128, 8*656] as moving: no ragged-i waste, bout added as
    per-partition bias during PSUM evacuation; host undoes the transpose.
  - Per-group software pipeline: group g's attention zips group g+1's
    projection chains and group g-1's final chains between dependency
    stages, keeping TensorE dense (full HAM clock).
"""

import os
import numpy as np
import ml_dtypes

import concourse.bass as bass
import concourse.bacc as bacc
import concourse.bass_isa as bass_isa
import concourse.mybir as mybir
import concourse.tile as tile
from concourse import bass_utils

BF16 = mybir.dt.bfloat16
F32 = mybir.dt.float32
F8 = mybir.dt.float8e4
AF = mybir.ActivationFunctionType
ALU = mybir.AluOpType
DR = mybir.MatmulPerfMode.DoubleRow
nbf16 = ml_dtypes.bfloat16
nf8 = ml_dtypes.float8_e4m3

B, N, D, H, NK, DH = 256, 164, 1024, 8, 100, 128
NCORES = 8
BPC = B // NCORES          # 32 batches per core
GB = 4                     # batches per group
NG = BPC // GB             # 8 groups
XC = GB * N                # 656 tokens per group
N1 = N - 128               # 36
MASK_NEG = -1.0e30
SQK = float(2 ** 15)       # Sq*Sk fp8 pre-scale (descaled at activation)

_CACHE = {}
LAST_EXEC_NS = None


def _install_profile_hook():
    """Make run_bass_kernel_spmd(trace=True) work under axon in this image."""
    import sys as _sys
    import types as _types
    try:
        import antenv  # noqa: F401
        try:
            from antenv.axon_hooks import get_axon_ntff_profile_hook  # noqa: F401
        except ImportError:
            from trn_agent_boot.trn_boot import _ntff_profile_via_ctypes
            hook = _ntff_profile_via_ctypes("/opt/axon/libaxon_pjrt.so")
            mod = _types.ModuleType("antenv.axon_hooks")
            mod._hook = hook
            mod.set_axon_ntff_profile_hook = lambda h: setattr(mod, "_hook", h)
            mod.get_axon_ntff_profile_hook = lambda: mod._hook
            _sys.modules["antenv.axon_hooks"] = mod
            antenv.axon_hooks = mod
        if not getattr(bass_utils, "_upload_patched", False):
            _orig_upload = bass_utils.upload_artifacts

            def _safe_upload(tmpdir):
                try:
                    return _orig_upload(tmpdir)
                except Exception:
                    return tmpdir

            bass_utils.upload_artifacts = _safe_upload
            bass_utils._upload_patched = True
        return True
    except Exception as e:  # pragma: no cover
        print(f"profile hook install failed: {type(e).__name__}: {e}")
        return False


def _build_nc():
    nc = bacc.Bacc("TRN2", target_bir_lowering=False, debug=False)

    # ---- DRAM parameters (per-core shapes) ----
    d_xtv = nc.dram_tensor("xtv", [NG, 128, 8, 768], BF16, kind="ExternalInput")
    d_xt8 = nc.dram_tensor("xt8", [NG, 128, 8, XC], F8, kind="ExternalInput")
    d_wqk8 = nc.dram_tensor("wqk8", [128, 8, 2 * D], F8, kind="ExternalInput")
    d_wv = nc.dram_tensor("wv", [128, 8, D], BF16, kind="ExternalInput")
    d_wout = nc.dram_tensor("wout", [128, 8 * D], BF16, kind="ExternalInput")
    d_w1 = nc.dram_tensor("w1", [NK, 50], BF16, kind="ExternalInput")
    d_w2 = nc.dram_tensor("w2", [50, NK], BF16, kind="ExternalInput")
    d_b1 = nc.dram_tensor("b1c", [50, 1], F32, kind="ExternalInput")
    d_b2 = nc.dram_tensor("b2c", [NK, 1], F32, kind="ExternalInput")
    d_boutT = nc.dram_tensor("boutT", [128, 8], F32, kind="ExternalInput")
    d_mbt0 = nc.dram_tensor("mbt0", [128, BPC], F32, kind="ExternalInput")
    d_mbt1 = nc.dram_tensor("mbt1", [N1, BPC], F32, kind="ExternalInput")
    d_xian2 = nc.dram_tensor("xian2", [NG, NK, 2 * GB, NK], BF16, kind="ExternalInput")
    d_y = nc.dram_tensor("y", [8, 128, BPC * N], BF16, kind="ExternalOutput")

    from contextlib import ExitStack
    with tile.TileContext(nc) as tc, ExitStack() as es:
        ec = es.enter_context
        cpool = ec(tc.tile_pool(name="const", bufs=1))
        xt_pool = ec(tc.tile_pool(name="xt", bufs=2))
        xt8_pool = ec(tc.tile_pool(name="xt8", bufs=2))
        xian_pool = ec(tc.tile_pool(name="xian", bufs=2))
        qk_pool = ec(tc.tile_pool(name="qk", bufs=2))
        v_pool = ec(tc.tile_pool(name="vsb", bufs=2))
        vtmp_pool = ec(tc.tile_pool(name="vtmp", bufs=2))
        outT_pool = ec(tc.tile_pool(name="outT", bufs=2))
        y_pool = ec(tc.tile_pool(name="ysb", bufs=2))
        rbc_pool = ec(tc.tile_pool(name="rbc", bufs=2))
        small_pool = ec(tc.tile_pool(name="smallsb", bufs=3))
        # PSUM is 8 banks; every tile slot is a full bank. 3 (proj ring)
        # + 2 (score-full) + 1 (score-tail + denom share a ring) + 2
        # (m1/m2/oT share a ring) = 8.
        pj = ec(tc.tile_pool(name="pproj", bufs=3, space="PSUM"))
        pscf = ec(tc.tile_pool(name="pscf", bufs=2, space="PSUM"))
        psct = ec(tc.tile_pool(name="psct", bufs=1, space="PSUM"))
        pmix = ec(tc.tile_pool(name="pmix", bufs=2, space="PSUM"))
        if True:
            # ---- constants ----
            wqk8_sb = cpool.tile([128, 8, 2 * D], F8, tag="wqk8")
            wv_sb = cpool.tile([128, 8, D], BF16, tag="wv")
            wout_sb = cpool.tile([128, 8 * D], BF16, tag="wout")
            w1_sb = cpool.tile([NK, 50], BF16, tag="w1")
            w2_sb = cpool.tile([50, NK], BF16, tag="w2")
            b1_sb = cpool.tile([50, 1], F32, tag="b1")
            b2_sb = cpool.tile([NK, 1], F32, tag="b2")
            boutT_sb = cpool.tile([128, 8], F32, tag="boutT")
            mbt0_sb = cpool.tile([128, BPC], F32, tag="mbt0")
            mbt1_sb = cpool.tile([N1, BPC], F32, tag="mbt1")
            onesm_sb = cpool.tile([128, 128], BF16, tag="onesm")
            nc.vector.memset(onesm_sb[:], 1.0)


            def load_consts():
                # wqk8 split per channel-tile pair so early qk chains can start
                for ct in range(8):
                    nc.sync.dma_start(
                        wqk8_sb[:, :, ct * 256:(ct + 1) * 256],
                        d_wqk8.ap()[:, :, ct * 256:(ct + 1) * 256])
                nc.sync.dma_start(wv_sb[:], d_wv.ap()[:, :, :])
                nc.sync.dma_start(w1_sb[:], d_w1.ap()[:, :])
                nc.sync.dma_start(w2_sb[:], d_w2.ap()[:, :])
                nc.sync.dma_start(b1_sb[:], d_b1.ap()[:, :])
                nc.sync.dma_start(b2_sb[:], d_b2.ap()[:, :])
                nc.sync.dma_start(mbt0_sb[:], d_mbt0.ap()[:, :])
                nc.sync.dma_start(mbt1_sb[:], d_mbt1.ap()[:, :])
                nc.sync.dma_start(wout_sb[:], d_wout.ap()[:, :])
                nc.sync.dma_start(boutT_sb[:], d_boutT.ap()[:, :])

            group_tiles = {}

            def start_group(g):
                """DMA group g's inputs; return list of projection closures."""
                xt8_sb = xt8_pool.tile([128, 8, XC], F8, tag="xt8")
                nc.sync.dma_start(xt8_sb[:], d_xt8.ap()[g, :, :, :])
                xtv_sb = xt_pool.tile([128, 8, 768], BF16, tag="xtv")
                nc.sync.dma_start(xtv_sb[:], d_xtv.ap()[g, :, :, :])
                xian_sb = xian_pool.tile([NK, 2 * GB, NK], BF16, tag="xian")
                nc.sync.dma_start(xian_sb[:], d_xian2.ap()[g, :, :, :])
                qkT = qk_pool.tile([128, 16 * XC], BF16, tag="qkT")
                v_sb = v_pool.tile([128, GB * 2 * D], BF16, tag="v")
                group_tiles[g] = (xtv_sb, xt8_sb, qkT, v_sb, xian_sb)

                def qk_chain(ct):
                    # fp8 DoubleRow: one stationary pair feeds both token
                    # halves; 4 chained matmuls cover the K=1024 contraction.
                    # Split into 2 sub-closures for finer zip interleaving.
                    st8 = {}

                    def half(lo):
                        if lo == 0:
                            st8["pa"] = pj.tile([128, 512], F32, tag="pj", name="qkpa")
                            st8["pb"] = pj.tile([128, 512], F32, tag="pj", name="qkpb")
                        pa, pb = st8["pa"], st8["pb"]
                        for p in range(lo, lo + 2):
                            st = wqk8_sb[:, 2 * p:2 * p + 2, ct * 128:ct * 128 + 128]
                            nc.tensor.matmul(
                                pa[:, :], st, xt8_sb[:, 2 * p:2 * p + 2, 0:512],
                                start=(p == 0), stop=(p == 3), perf_mode=DR)
                            nc.tensor.matmul(
                                pb[:, :144], st, xt8_sb[:, 2 * p:2 * p + 2, 512:XC],
                                start=(p == 0), stop=(p == 3), perf_mode=DR)
                        if lo == 2:
                            nc.scalar.activation(qkT[:, ct * XC:ct * XC + 512],
                                                 pa[:], AF.Copy)
                            nc.scalar.activation(qkT[:, ct * XC + 512:ct * XC + XC],
                                                 pb[:, :144], AF.Copy)
                    return [lambda: half(0), lambda: half(2)]

                def v_chain(s):
                    # xtv stationary tiles: s in 0..3 are batch s tokens
                    # 0:128; s==4 is the packed tails (b0,b1,b2 full + b3's
                    # first 20); s==5 is b3's last 16 tail tokens (+zero pad).
                    # Split into 4 sub-closures of 2 dt-steps each.
                    stv = {}
                    pw = 128 if s < 5 else 16

                    def quarter(lo):
                        if lo == 0:
                            stv["pa"] = pj.tile([128, 512], F32, tag="pj", name="vpa")
                            stv["pb"] = pj.tile([128, 512], F32, tag="pj", name="vpb")
                        pa, pb = stv["pa"], stv["pb"]
                        for dt in range(lo, lo + 2):
                            st = xtv_sb[:, dt, s * 128:s * 128 + 128]
                            nc.tensor.matmul(pa[:pw, :], st[:, :pw],
                                             wv_sb[:, dt, 0:512],
                                             start=(dt == 0), stop=(dt == 7))
                            nc.tensor.matmul(pb[:pw, :], st[:, :pw],
                                             wv_sb[:, dt, 512:D],
                                             start=(dt == 0), stop=(dt == 7))
                        if lo == 6:
                            v_evac(s, pa, pb, pw)

                    return [lambda q=q: quarter(2 * q) for q in range(4)]

                def v_evac(s, pa, pb, pw):
                    if s < 4:
                        base = (s * 2) * D
                        nc.vector.tensor_copy(v_sb[:128, base:base + 512], pa[:128, :])
                        nc.vector.tensor_copy(v_sb[:128, base + 512:base + D],
                                              pb[:128, :])
                    elif s == 4:
                        vt = vtmp_pool.tile([128, D], BF16, tag="vt")
                        nc.vector.tensor_copy(vt[:, 0:512], pa[:, :])
                        nc.vector.tensor_copy(vt[:, 512:D], pb[:, :])
                        # scatter packed tails to per-batch homes
                        for bb in range(3):
                            nc.sync.dma_start(
                                v_sb[0:N1, (bb * 2 + 1) * D:(bb * 2 + 2) * D],
                                vt[bb * N1:(bb + 1) * N1, :])
                        nc.sync.dma_start(
                            v_sb[0:20, 7 * D:8 * D], vt[108:128, :])
                    else:
                        vt = vtmp_pool.tile([128, D], BF16, tag="vt2")
                        nc.vector.tensor_copy(vt[:16, 0:512], pa[:16, :])
                        nc.vector.tensor_copy(vt[:16, 512:D], pb[:16, :])
                        nc.sync.dma_start(
                            v_sb[20:N1, 7 * D:8 * D], vt[0:16, :])

                qk_parts, v_parts = [], []
                for i in range(8):
                    qk_parts += qk_chain(i)
                    qk_parts += qk_chain(8 + i)
                    if i < 6:
                        v_parts += v_chain(i)
                chains = []
                for i in range(8):
                    chains += qk_parts[4 * i:4 * i + 4]
                    if i < 6:
                        chains += v_parts[4 * i:4 * i + 4]
                # prologue variant: qk first (xt8 lands before xtv)
                return chains, qk_parts + v_parts

            def final_chain(dout, outT_prev, gp):
                # split into 4 sub-closures of 2 h-steps each
                stf = {}

                def part(lo):
                    if lo == 0:
                        stf["pa"] = pj.tile([128, 512], F32, tag="pj", name="fpa")
                        stf["pb"] = pj.tile([128, 512], F32, tag="pj", name="fpb")
                    pa, pb = stf["pa"], stf["pb"]
                    for h in range(lo, lo + 2):
                        st = wout_sb[:, h * D + dout * 128:h * D + dout * 128 + 128]
                        nc.tensor.matmul(pa[:, :328], st,
                                         outT_prev[:, h, 0:328],
                                         start=(h == 0), stop=(h == 7))
                        nc.tensor.matmul(pb[:, :328], st,
                                         outT_prev[:, h, 328:XC],
                                         start=(h == 0), stop=(h == 7))
                    if lo == 6:
                        y_sb = y_pool.tile([128, XC], BF16, tag="y")
                        nc.scalar.activation(y_sb[:, 0:328], pa[:, :328],
                                             AF.Identity,
                                             bias=boutT_sb[:, dout:dout + 1])
                        nc.scalar.activation(y_sb[:, 328:XC], pb[:, :328],
                                             AF.Identity,
                                             bias=boutT_sb[:, dout:dout + 1])
                        nc.sync.dma_start(
                            d_y.ap()[dout, :, gp * XC:(gp + 1) * XC], y_sb[:])

                return [lambda p=p: part(2 * p) for p in range(4)]

            def final_chain_b(dout, outT_prev, gp, b):
                # single-batch final (last group: overlap drain with attention)
                pa = pj.tile([128, 512], F32, tag="pj")
                for h in range(8):
                    st = wout_sb[:, h * D + dout * 128:h * D + dout * 128 + 128]
                    nc.tensor.matmul(pa[:, :N], st,
                                     outT_prev[:, h, b * N:b * N + N],
                                     start=(h == 0), stop=(h == 7))
                y_sb = y_pool.tile([128, N], BF16, tag="yb")
                nc.scalar.activation(y_sb[:], pa[:, :N], AF.Identity,
                                     bias=boutT_sb[:, dout:dout + 1])
                nc.sync.dma_start(
                    d_y.ap()[dout, :, gp * XC + b * N:gp * XC + (b + 1) * N],
                    y_sb[:])

            # ---- prologue ----
            # static probs ring: tail rows 36:128 (cols 328:656) are zeroed
            # once and never rewritten, so the Pool all-reduce over the full
            # tile sums full+tail parts without per-pair masking
            pc_ring = []
            for ri in range(3):
                pcr = cpool.tile([128, XC], BF16, tag=f"pc{ri}",
                                 name=f"pcr{ri}")
                nc.vector.memset(pcr[:], 0.0)
                pc_ring.append(pcr)
            pair_seq = [0]
            _, g0_chains = start_group(0)
            load_consts()
            for ch in g0_chains:
                ch()

            pending_final = None  # (outT, g)
            inv = 1.0 / SQK

            for g in range(NG):
                xtv_sb, xt8_sb, qkT, v_sb, xian_sb = group_tiles.pop(g)
                zlist = []
                if g + 1 < NG:
                    zlist += start_group(g + 1)[0]
                if pending_final is not None:
                    of, gp = pending_final
                    for dout in range(8):
                        zlist += final_chain(dout, of, gp)
                pending_final = None

                outT = outT_pool.tile([128, 8, XC], BF16, tag="outT")
                zi = 0          # next zip item
                steps = 0       # zip-points passed (6 per pair)
                ztot = 6 * GB * 4

                def zip_step():
                    nonlocal zi, steps
                    steps += 1
                    while zi < (len(zlist) * steps) // ztot:
                        zlist[zi]()
                        zi += 1

                for b in range(GB):
                    gb = g * GB + b
                    for hp in range(4):
                        h0 = 2 * hp
                        # ---- scores (PSUM-resident) ----
                        scf = pscf.tile([128, 2, N], F32, tag="scf")
                        sct = psct.tile([128, 2 * N], F32, tag="sct")
                        for i in range(2):
                            qof = (h0 + i) * XC + b * N
                            kof = (8 + h0 + i) * XC + b * N
                            nc.tensor.matmul(scf[:, i, :], qkT[:, kof:kof + 128],
                                             qkT[:, qof:qof + N])
                            nc.tensor.matmul(sct[:N1, i * N:i * N + N],
                                             qkT[:, kof + 128:kof + N],
                                             qkT[:, qof:qof + N])
                        zip_step()
                        # ---- keypoint MLP (both heads batched) ----
                        raq = small_pool.tile([NK, 2, NK], BF16, tag="raq")
                        nc.scalar.activation(raq[:], scf[:NK, :, :NK], AF.Relu,
                                             scale=inv)
                        mx1 = pmix.tile([128, 2 * N], F32, tag="mix")
                        m1 = mx1[:50, :2 * NK]
                        nc.tensor.matmul(m1, w1_sb[:], raq[:])
                        zip_step()
                        h1t = small_pool.tile([50, 2 * NK], BF16, tag="h1")
                        nc.scalar.activation(h1t[:], m1, AF.Relu,
                                             bias=b1_sb[:])
                        mx2 = pmix.tile([128, 2 * N], F32, tag="mix")
                        m2 = mx2[:NK, :2 * NK]
                        nc.tensor.matmul(m2, w2_sb[:], h1t[:])
                        zip_step()
                        lvt = small_pool.tile([NK, 2 * NK], BF16, tag="lv")
                        nc.vector.tensor_scalar(lvt[:], m2, b2_sb[:], 0.0,
                                                ALU.add, ALU.max)
                        tmp = small_pool.tile([NK, 2 * NK], F32, tag="tmp")
                        nc.vector.tensor_mul(tmp[:], xian_sb[:, 2 * b:2 * b + 2, :],
                                             lvt[:])
                        nc.vector.tensor_add(scf[:NK, :, :NK], scf[:NK, :, :NK],
                                             tmp[:])
                        # ---- exp straight out of PSUM (mask bias, descale);
                        # full probs in cols 0:328, tail probs in cols
                        # 328:656 rows 0:36 (rows 36:128 pre-zeroed) ----
                        pc = pc_ring[pair_seq[0] % 3]
                        pair_seq[0] += 1
                        nc.scalar.activation(pc[:, 0:2 * N], scf[:], AF.Exp,
                                             bias=mbt0_sb[:, gb:gb + 1], scale=inv)
                        nc.scalar.activation(pc[0:N1, 2 * N:XC], sct[:N1], AF.Exp,
                                             bias=mbt1_sb[:, gb:gb + 1], scale=inv)
                        # ---- denominator (ones-matmul partition broadcast);
                        # keep <=2 exp-waiters queued on PE at a time so the
                        # depth-4 wait queue never clogs ----
                        db = psct.tile([128, 2 * N], F32, tag="sct")
                        nc.tensor.matmul(db[:], onesm_sb[:], pc[:, 0:2 * N],
                                         start=True, stop=False)
                        nc.tensor.matmul(db[:], onesm_sb[:N1, :],
                                         pc[0:N1, 2 * N:XC],
                                         start=False, stop=True)
                        rb = rbc_pool.tile([128, 2 * N], F32, tag="rb")
                        nc.vector.reciprocal_approx_fast(rb[:], db[:])
                        zip_step()
                        # ---- AV ----
                        ot = pmix.tile([128, 2 * N], F32, tag="mix")
                        for i in range(2):
                            h = h0 + i
                            nc.tensor.matmul(
                                ot[:, i * N:i * N + N],
                                v_sb[:, (b * 2) * D + h * DH:(b * 2) * D + h * DH + DH],
                                pc[:, i * N:i * N + N], start=True, stop=False)
                            nc.tensor.matmul(
                                ot[:, i * N:i * N + N],
                                v_sb[:N1, (b * 2 + 1) * D + h * DH:(b * 2 + 1) * D + h * DH + DH],
                                pc[0:N1, 2 * N + i * N:2 * N + i * N + N],
                                start=False, stop=True)
                            if i == 0:
                                zip_step()
                        nc.vector.tensor_mul(
                            outT[:, h0:h0 + 2, b * N:b * N + N], ot[:], rb[:])
                        zip_step()

                    if g == NG - 1:
                        for dout in range(8):
                            final_chain_b(dout, outT, g, b)

                while zi < len(zlist):
                    zlist[zi]()
                    zi += 1
                if g < NG - 1:
                    pending_final = (outT, g)

    nc.compile()
    return nc


def _prep_core_inputs(xc, maskc, xianc, consts):
    # (g, p, dt, b, n) layout, partition = d-within-dt
    xt = xc.transpose(0, 2, 1).reshape(NG, GB, 8, 128, N)
    xt = np.ascontiguousarray(xt.transpose(0, 3, 2, 1, 4))
    # xtv bf16: v-projection stationary tiles [NG, 128, 8, 6*128]:
    # tiles 0..3 = per-batch tokens 0:128; tile 4 = packed tails
    # (b0,b1,b2 all 36 + b3's first 20); tile 5 = b3 tail 20:36 + zeros.
    xtv = np.zeros((NG, 128, 8, 768), dtype=np.float32)
    for bb in range(GB):
        xtv[:, :, :, bb * 128:(bb + 1) * 128] = xt[:, :, :, bb, 0:128]
    tails = xt[:, :, :, :, 128:N]                    # [NG,128,8,GB,36]
    packed = tails.reshape(NG, 128, 8, GB * N1)      # b-major 144
    xtv[:, :, :, 512:640] = packed[:, :, :, 0:128]
    xtv[:, :, :, 640:656] = packed[:, :, :, 128:144]
    xtv = xtv.astype(nbf16)
    # xt8 fp8: [NG, 128, 8, 656] (tokens contiguous across batches)
    xt8 = xt.reshape(NG, 128, 8, XC).astype(nf8)
    # mask bias transposed: [164, BPC]
    mb = np.where(maskc, np.float32(MASK_NEG), np.float32(0.0)).astype(np.float32)
    mbt = np.ascontiguousarray(mb.T)
    # xian2: xianT scaled by SQK, duplicated per head-pair lane:
    # [NG, NK(j), 2*GB, NK(i)] where dim2 = (b, dup)
    xiant = (xianc.transpose(0, 2, 1) * np.float32(SQK)).reshape(NG, GB, NK, NK)
    xian2 = np.empty((NG, NK, GB, 2, NK), dtype=np.float32)
    xian2[:, :, :, 0, :] = xiant.transpose(0, 2, 1, 3)
    xian2[:, :, :, 1, :] = xiant.transpose(0, 2, 1, 3)
    xian2 = np.ascontiguousarray(xian2.reshape(NG, NK, 2 * GB, NK)).astype(nbf16)
    out = {
        "xtv": xtv,
        "xt8": xt8,
        "mbt0": np.ascontiguousarray(mbt[:128]),
        "mbt1": np.ascontiguousarray(mbt[128:]),
        "xian2": xian2,
    }
    out.update(consts)
    return out


def kernel(x, mask, xian, Wqkv, W1, b1, W2, b2, Wout, bout):
    global LAST_EXEC_NS
    x = np.asarray(x, dtype=np.float32)
    mask = np.asarray(mask)
    xian = np.asarray(xian, dtype=np.float32)
    Wqkv = np.asarray(Wqkv, dtype=np.float32)
    W1 = np.asarray(W1, dtype=np.float32)
    b1 = np.asarray(b1, dtype=np.float32)
    W2 = np.asarray(W2, dtype=np.float32)
    b2 = np.asarray(b2, dtype=np.float32)
    Wout = np.asarray(Wout, dtype=np.float32)
    bout = np.asarray(bout, dtype=np.float32)

    if "nc" not in _CACHE:
        _CACHE["nc"] = _build_nc()
    nc = _CACHE["nc"]

    # ---- shared weight prep ----
    scale = np.float32(D ** -0.5)
    # fp8 q/k weights with power-of-two pre-scales (descaled on device)
    wq = Wqkv[:, :D] * (scale * np.float32(1024.0))
    wk = Wqkv[:, D:2 * D] * np.float32(32.0)
    wqk = np.concatenate([wq, wk], axis=1)                    # [D, 2D]
    wqk8 = np.ascontiguousarray(
        wqk.reshape(8, 128, 2 * D).transpose(1, 0, 2)).astype(nf8)
    wv_h = np.ascontiguousarray(
        Wqkv[:, 2 * D:].reshape(8, 128, D).transpose(1, 0, 2)).astype(nbf16)
    wout_h = np.ascontiguousarray(
        Wout.reshape(8, 128, D).transpose(1, 0, 2)).reshape(128, 8 * D).astype(nbf16)
    consts = {
        "wqk8": wqk8,
        "wv": wv_h,
        "wout": wout_h,
        "w1": W1.astype(nbf16),
        "w2": W2.astype(nbf16),
        "b1c": np.ascontiguousarray(b1.reshape(50, 1)),
        "b2c": np.ascontiguousarray(b2.reshape(NK, 1)),
        "boutT": np.ascontiguousarray(bout.reshape(8, 128).T).astype(np.float32),
    }

    in_maps = []
    for c in range(NCORES):
        sl = slice(c * BPC, (c + 1) * BPC)
        in_maps.append(_prep_core_inputs(x[sl], mask[sl], xian[sl], consts))

    trace = bool(int(os.environ.get("KERNEL_TRACE", "0")))
    if trace:
        trace = _install_profile_hook()
    res = bass_utils.run_bass_kernel_spmd(
        nc, in_maps, core_ids=list(range(NCORES)), trace=trace)
    LAST_EXEC_NS = res.exec_time_ns
    _CACHE["res"] = res

    out = np.empty((B, N, D), dtype=np.float32)
    for c in range(NCORES):
        yc = res.results[c]["y"].astype(np.float32)           # [8, 128, BPC*N]
        yc = yc.reshape(8, 128, BPC, N).transpose(2, 3, 0, 1).reshape(BPC, N, D)
        out[c * BPC:(c + 1) * BPC] = yc
    return out
